# revision 19
# baseline (speedup 1.0000x reference)
"""Trainium2 Bass kernel for nn_GatedBlock (moe_routing).

Math: out[b, i] = g[b, i // 128] * (x @ W.T)[b, i] + bias[i]
with g = sigmoid(x @ gate_w + gate_b), bottom-8 of 16 gates zeroed per row.

Design (vs the 24.9us f32r baseline; ~18.0-18.5us measured):
  * dtypes: one fp16 stationary x.T/64 serves both the gate matmuls (vs
    fp16 gate_w*64) and the main matmuls (vs e3m4 W*64).  The /64,*64
    scales are exact exponent shifts folded on the host.  Gate-linear
    precision (11-bit mantissa operands) matches the f32r baseline - the
    top-8 selection margin (3.4e-4) survives; main-path rel err ~1.1e-2
    vs the 2e-2 gate.  743KB/core of HBM traffic instead of 2.5MB.
  * DMA: three transfers on the sync HWDGE queue (pre16, rhs[0:8],
    rhs[8:16]) with >=1.5KB per-partition descriptor runs (the effective
    rate ceiling ~200GB/s saturates there); epi on scalar (its hoisted
    ACT table loads delay nothing late-needed); out halves split
    sync/scalar so the HBM-write receipts overlap.  No gpsimd/SWDGE: its
    descriptor-ring traffic contends with SDMA engines 7/15 and delays
    every queue's completion semaphore.
  * PE: fp16 x fp8e3m4 mixed matmuls (1 cycle/col, cheap weight loads),
    2-way column-tiled via tile_position (M=32 occupies a quarter of the
    128-wide array).  HSPLIT: each matmul covers one 128-col output half
    (N=128, 4 chains into per-half psum tiles in separate banks), emitted
    h0-before-h1 per chunk, so half 0's epilogue and out-dma overlap
    half 1's final matmuls.  Gate matmuls sit at col positions (0,64),
    (0,96), never sharing array columns with the main groups.
  * epilogue: out[:, h] = (ps0 + ps1)*gk[h] + bias as two chained
    scalar_tensor_tensor passes per half (walrus allows one PSUM input
    per DVE op; 32-partition ops may read any 32-aligned window); each
    half's out-DMA issues as soon as its chain finishes.
  * psum tiles padded to full 2KB banks so concurrent PE-writes and
    DVE-reads never share a bank.
"""

import sys

for _p in ("/opt/trn_rl_repo", "/root/.axon_site/_ro/trn_rl_repo"):
    if _p not in sys.path:
        sys.path.append(_p)

import os as _os

import numpy as np

B = 32          # batch
D = 2048        # model dim
NB = 16         # gate blocks
BLK = D // NB   # 128 output rows per gate block
N_CORES = 8
NOUT = D // N_CORES       # 256 output cols per core
KT = D // 128             # 16 k-tiles
SCALE = 64.0              # x/SCALE fp16, W*SCALE e3m4, gate_w*SCALE fp16

# rhs chunking: "s" chunks issue on sync after pre16, "c" chunks on scalar
# after epi.  Format: engine:lo:hi per chunk.  No gpsimd/SWDGE: its
# descriptor-ring traffic contends with SDMA engines 7/15 and makes every
# queue's completion semaphore late.
CHUNKS = [c.split(":") for c in
          _os.environ.get("GATED2_CHUNKS", "s:0:8,s:8:16").split(",")]
CHUNKS = [(e, int(lo), int(hi)) for e, lo, hi in CHUNKS]
# matmul emission order: chunks sorted by this key (expected arrival order)
_order_env = _os.environ.get("GATED2_ORDER", "")
CHUNK_ORDER = ([int(v) for v in _order_env.split(",")] if _order_env
               else list(range(len(CHUNKS))))
GATE_4WAY = _os.environ.get("GATED2_GATE4", "0") == "1"
PRE_SPLIT = int(_os.environ.get("GATED2_PRE_SPLIT", "1"))
SPLIT_OUT = _os.environ.get("GATED2_SPLIT_OUT", "1") == "1"
PRE_ENG = _os.environ.get("GATED2_PRE_ENG", "s")  # s=sync, c=scalar
# pre16 k-tile ranges per ring ("s:0:8,c:8:16" puts tiles 0-7 on sync and
# 8-15 on scalar).  Empty = legacy PRE_ENG/PRE_SPLIT behavior.
_pre_env = _os.environ.get("GATED2_PRE_CHUNKS", "")
PRE_CHUNKS = ([(e, int(lo), int(hi)) for e, lo, hi in
               (c.split(":") for c in _pre_env.split(","))] if _pre_env
              else None)
# gate matmul col positions (comma list).  q64/q96 keeps gates off the
# main groups q0/q32; "0,32" frees q64/q96 for long-running warm dummies.
GATE_POS = [int(v) for v in
            _os.environ.get("GATED2_GATE_POS", "64,96").split(",")]
# warm dummy col positions (cycled).  Dummies at q64/q96 never block the
# main chains at q0/q32, so the warm stream can safely outlast the DMA.
WARM_POS = [int(v) for v in
            _os.environ.get("GATED2_WARM_POS", "0").split(",")]
# where the epi (bias+gate_b) load sits in the scalar ring order
EPI_LAST = _os.environ.get("GATED2_EPI_LAST", "0") == "1"
# scheme e: gate chains duplicated on partition groups 0-31/32-63 (full
# k-sum each), mains one chain per half into a single [64,128] psum tile,
# epilogue = 2 tiny gate-col selects + ONE 64-partition DVE stt, out-dma
# halves from partition ranges 0-31 (sync) / 32-63 (scalar).
SCHEME = _os.environ.get("GATED2_SCHEME", "old")
# hoist the input dma_starts from the kernel body into the framework
# preamble (right after each engine's register setup), so the HWDGE rings
# start pulling input while the other engines still run their ~7us entry
# sequence.  Inputs are in HBM before any NEFF instruction executes, and
# the completion sems were zeroed by the previous execution's teardown.
HOIST = int(_os.environ.get("GATED2_HOIST", "0"))  # 0=off, 1=at preamble_end,
# 2=before the engine's TPBBaseLd (executes right after the entry barrier),
# 3=at the engine's stream head
# HAM warmup: dummy matmuls on a memset tile keep the PE busy from the end
# of the framework prologue until the first real data lands, so the PE
# clock-gate opens (1.2 -> 2.4 GHz) before the real matmuls run.
WARM = int(_os.environ.get("GATED2_WARM", "0"))
# free-dim of each warmup matmul (cold cost ~ N/1.2 ns each)
WARM_N = int(_os.environ.get("GATED2_WARM_N", "512"))
# epilogue engine per output half: v=vector(DVE), g=gpsimd(Pool).  "vg"
# runs half 0 on DVE and half 1 on Pool concurrently.
EPI_ENG = _os.environ.get("GATED2_EPI_ENG", "vv")
# half-split main matmuls: N=128 per matmul, 4 chains (2 col-groups x 2
# output halves) into per-half psum tiles (separate banks), emitted
# h0-before-h1 per chunk so half 0's epilogue + out-dma overlap half 1's
# final matmuls.
HSPLIT = _os.environ.get("GATED2_HSPLIT", "1") == "1"

_compiled = {}


def _build_e(key):
    """Scheme e: single [64,128] main psum (one chain per half at q0/q32),
    gate chains duplicated on both partition groups (full k-sum each), one
    64-partition epilogue stt, out halves DMA'd from partition ranges."""
    import concourse.bacc as bacc
    import concourse.tile as tile
    import concourse.mybir as mybir

    f32 = mybir.dt.float32
    f16 = mybir.dt.float16
    f8 = mybir.dt.float8e3

    nc = bacc.Bacc("TRN2", target_bir_lowering=False, debug=False,
                   num_devices=N_CORES)

    pre_d = nc.dram_tensor("pre16", [128, KT, B + NB], f16, kind="ExternalInput")
    rhs_d = nc.dram_tensor("rhs8", [128, KT, NOUT], f8, kind="ExternalInput")
    epi_d = nc.dram_tensor("epi", [64, BLK + NB], f32, kind="ExternalInput")
    out_d = nc.dram_tensor("out", [B, NOUT], f32, kind="ExternalOutput")

    with tile.TileContext(nc) as tc:
        with (
            tc.tile_pool(name="sb", bufs=1) as sb,
            tc.tile_pool(name="ps", bufs=1, space="PSUM") as psp,
        ):
            pre = sb.tile([128, KT, B + NB], f16, name="pre_sb", tag="pre_sb")
            rhs = sb.tile([128, KT, NOUT], f8, name="rhs_sb", tag="rhs_sb")
            epi = sb.tile([64, BLK + NB], f32, name="epi_sb", tag="epi_sb")
            g = sb.tile([64, NB], f32, name="g", tag="g")
            gs = sb.tile([64, NB], f32, name="gs", tag="gs")
            m8 = sb.tile([64, 8], f32, name="m8", tag="m8")
            rep = sb.tile([64, NB], f32, name="rep", tag="rep")
            gk = sb.tile([64, NB], f32, name="gk", tag="gk")
            gsel = sb.tile([64, 1], f32, name="gsel", tag="gsel")
            outt = sb.tile([64, BLK], f32, name="outt", tag="outt")

            ps_g = psp.tile([64, NB], f32, name="ps_g", tag="ps_g",
                            padded_shape=[64, 512])
            ps_m = psp.tile([64, BLK], f32, name="ps_m", tag="ps_m",
                            padded_shape=[64, 512])

            # ---- DMA issues ----
            in_dmas = []
            if PRE_CHUNKS is not None:
                for eng, lo, hi in PRE_CHUNKS:
                    e = nc.sync if eng == "s" else nc.scalar
                    in_dmas.append(
                        e.dma_start(pre[:, lo:hi, :], pre_d.ap()[:, lo:hi, :]))
            else:
                pre_e = nc.sync if PRE_ENG == "s" else nc.scalar
                in_dmas.append(pre_e.dma_start(pre[:], pre_d.ap()))
            if not EPI_LAST:
                in_dmas.append(nc.scalar.dma_start(epi[:], epi_d.ap()))
            for eng, lo, hi in CHUNKS:
                e = nc.sync if eng == "s" else nc.scalar
                in_dmas.append(
                    e.dma_start(rhs[:, lo:hi, :], rhs_d.ap()[:, lo:hi, :]))
            if EPI_LAST:
                in_dmas.append(nc.scalar.dma_start(epi[:], epi_d.ap()))

            # ---- HAM warmup: dummies at q64/q96 never touch q0/q32 ----
            warm_insts = []
            if WARM > 0:
                dum = sb.tile([128, WARM_N], f16, name="dum", tag="dum")
                ps_d = psp.tile([128, WARM_N], f32, name="ps_d", tag="ps_d",
                                padded_shape=[128, 512])
                if WARM_PRE == 0:
                    # body mode: zero the operand tile first
                    nc.gpsimd.memset(dum[:], 0.0)
                for i in range(WARM):
                    p = 64 + 32 * (i % 2)
                    warm_insts.append(
                        nc.tensor.matmul(ps_d[p:p + B, :], dum[:, :B], dum[:],
                                         start=True, stop=True,
                                         tile_position=(0, p)))

            # ---- gate matmuls: two full-k chains at q0 and q32 ----
            for t in range(KT):
                for p in (0, 32):
                    nc.tensor.matmul(
                        ps_g[p:p + B, :],
                        pre[:, t, :B], pre[:, t, B:B + NB],
                        start=(t == 0), stop=(t == KT - 1),
                        tile_position=(0, p),
                    )

            # gate pipeline on 64 partitions (same op count as 32)
            nc.vector.tensor_add(gs[:], ps_g[0:64, :], epi[:, BLK:BLK + NB])
            nc.scalar.activation(g[:], gs[:],
                                 mybir.ActivationFunctionType.Sigmoid)
            nc.vector.max(m8[:], g[:])
            nc.vector.match_replace(rep[:], m8[:], g[:], 0.0)
            nc.vector.tensor_sub(gk[:], g[:], rep[:])
            # per-half gate column select (ACT engine, off the DVE path)
            nc.scalar.copy(gsel[0:B, :], gk[0:B, 0:1])
            nc.scalar.copy(gsel[B:64, :], gk[B:64, 1:2])

            # ---- main matmuls: one chain per half, h0@q0 / h1@q32 ----
            order = []
            for ci in CHUNK_ORDER:
                _, lo, hi = CHUNKS[ci]
                order += list(range(lo, hi))
            assert sorted(order) == list(range(KT)), order
            for ci in CHUNK_ORDER:
                _, lo, hi = CHUNKS[ci]
                for t in range(lo, hi):
                    for h in range(2):
                        nc.tensor.matmul(
                            ps_m[32 * h:32 * h + B, :],
                            pre[:, t, :B],
                            rhs[:, t, h * BLK:(h + 1) * BLK],
                            start=(t == order[0]), stop=(t == order[-1]),
                            tile_position=(0, 32 * h),
                        )

            # ---- epilogue: ONE 64-partition stt, then split out-dma ----
            nc.vector.scalar_tensor_tensor(
                outt[:], ps_m[0:64, :], gsel[:, 0:1], epi[:, 0:BLK],
                mybir.AluOpType.mult, mybir.AluOpType.add)
            nc.sync.dma_start(out_d.ap()[:, 0:BLK], outt[0:B, :])
            nc.scalar.dma_start(out_d.ap()[:, BLK:NOUT], outt[B:64, :])

    if HOIST > 0:
        _hoist_input_dmas(nc, in_dmas)

    nc.compile()
    return nc


def _hoist_input_dmas(nc, in_dmas):
    """Move the input dma_start instructions from the body block into the
    preamble block, right after each issuing engine's register setup, so
    the loads issue at ~t=0.1-1.4us instead of ~7us."""
    entry = nc.main_func.blocks[0]
    per_eng = {}
    for h in in_dmas:
        inst = h.ins
        blk = None
        for b in nc.main_func.blocks:
            if any(x is inst for x in b.instructions):
                blk = b
                break
        assert blk is not None and blk is not entry, "input dma not in body"
        blk.instructions[:] = [x for x in blk.instructions if x is not inst]
        per_eng.setdefault(inst.engine, []).append(inst)
    for engkey, insts in per_eng.items():
        stream = nc.engines[engkey]
        assert stream.preamble_end is not None
        if HOIST == 1:
            idx = next(i for i, x in enumerate(entry.instructions)
                       if x is stream.preamble_end) + 1
        elif HOIST == 2:
            idx = next(i for i, x in enumerate(entry.instructions)
                       if type(x).__name__ == "InstTPBBaseLd"
                       and x.engine == engkey)
        else:
            idx = next(i for i, x in enumerate(entry.instructions)
                       if getattr(x, "engine", None) == engkey)
        entry.instructions[idx:idx] = insts


def _build(key):
    import concourse.bacc as bacc
    import concourse.tile as tile
    import concourse.mybir as mybir

    f32 = mybir.dt.float32
    f16 = mybir.dt.float16
    f8 = mybir.dt.float8e3

    nc = bacc.Bacc("TRN2", target_bir_lowering=False, debug=False,
                   num_devices=N_CORES)

    pre_d = nc.dram_tensor("pre16", [128, KT, B + NB], f16, kind="ExternalInput")
    rhs_d = nc.dram_tensor("rhs8", [128, KT, NOUT], f8, kind="ExternalInput")
    epi_d = nc.dram_tensor("epi", [B, NOUT + NB], f32, kind="ExternalInput")
    out_d = nc.dram_tensor("out", [B, NOUT], f32, kind="ExternalOutput")

    with tile.TileContext(nc) as tc:
        with (
            tc.tile_pool(name="sb", bufs=1) as sb,
            tc.tile_pool(name="ps", bufs=1, space="PSUM") as psp,
        ):
            pre = sb.tile([128, KT, B + NB], f16, name="pre_sb", tag="pre_sb")
            rhs = sb.tile([128, KT, NOUT], f8, name="rhs_sb", tag="rhs_sb")
            epi = sb.tile([B, NOUT + NB], f32, name="epi_sb", tag="epi_sb")
            g0s = sb.tile([B, NB], f32, name="g0s", tag="g0s")
            g1s = sb.tile([B, NB], f32, name="g1s", tag="g1s")
            g2s = sb.tile([B, NB], f32, name="g2s", tag="g2s")
            graw = sb.tile([B, NB], f32, name="graw", tag="graw")
            g = sb.tile([B, NB], f32, name="g", tag="g")
            m8 = sb.tile([B, 8], f32, name="m8", tag="m8")
            rep = sb.tile([B, NB], f32, name="rep", tag="rep")
            gk = sb.tile([B, NB], f32, name="gk", tag="gk")
            tmp = sb.tile([B, NOUT], f32, name="tmp", tag="tmp")
            tmp2 = sb.tile([B, NOUT], f32, name="tmp2", tag="tmp2")
            outt = sb.tile([B, NOUT], f32, name="outt", tag="outt")

            # padded to full 2KB banks: PE-writes and DVE-reads of different
            # tiles never share a bank
            ps_g = psp.tile([128, NB], f32, name="ps_g", tag="ps_g",
                            padded_shape=[128, 512])
            if HSPLIT:
                psH = [psp.tile([64, BLK], f32, name=f"psH{h}", tag=f"psH{h}",
                                padded_shape=[64, 512]) for h in range(2)]
            else:
                psA = psp.tile([64, NOUT], f32, name="psA", tag="psA",
                               padded_shape=[64, 512])

            # ---- DMA issues ----
            # sync: pre16 (first - it gates all PE work; split so the gate
            # matmuls start half a transfer earlier), then its rhs chunks.
            # scalar: epi + its rhs chunk; the hoisted ACT table load delays
            # scalar's first issue by ~1.3us, so scalar carries only
            # late-needed data.
            if PRE_CHUNKS is not None:
                for eng, lo, hi in PRE_CHUNKS:
                    e = nc.sync if eng == "s" else nc.scalar
                    e.dma_start(pre[:, lo:hi, :], pre_d.ap()[:, lo:hi, :])
            else:
                pre_e = nc.sync if PRE_ENG == "s" else nc.scalar
                if PRE_SPLIT > 1:
                    step = (KT + PRE_SPLIT - 1) // PRE_SPLIT
                    for lo in range(0, KT, step):
                        hi = min(lo + step, KT)
                        pre_e.dma_start(pre[:, lo:hi, :],
                                        pre_d.ap()[:, lo:hi, :])
                else:
                    pre_e.dma_start(pre[:], pre_d.ap())
            if not EPI_LAST:
                nc.scalar.dma_start(epi[:], epi_d.ap())
            for eng, lo, hi in CHUNKS:
                e = nc.sync if eng == "s" else nc.scalar
                e.dma_start(rhs[:, lo:hi, :], rhs_d.ap()[:, lo:hi, :])
            if EPI_LAST:
                nc.scalar.dma_start(epi[:], epi_d.ap())

            # ---- HAM warmup ----
            if WARM > 0:
                dum = sb.tile([128, WARM_N], f16, name="dum", tag="dum")
                ps_d = psp.tile([128, WARM_N], f32, name="ps_d", tag="ps_d",
                                padded_shape=[128, 512])
                nc.gpsimd.memset(dum[:], 0.0)
                for i in range(WARM):
                    p = WARM_POS[i % len(WARM_POS)]
                    nc.tensor.matmul(ps_d[p:p + B, :], dum[:, :B], dum[:],
                                     start=True, stop=True,
                                     tile_position=(0, p))

            # ---- gate matmuls ----
            # col positions (0,64),(0,96) [+(0,0),(0,32) in 4-way mode]:
            # gate output partitions sit at 64..127 so the reduce reads
            # 32-aligned windows there and main groups 0/1 are untouched.
            gpos = ([64, 96, 0, 32] if GATE_4WAY else list(GATE_POS))
            ng = len(gpos)
            for t in range(KT):
                j = t % ng
                p = gpos[j]
                nc.tensor.matmul(
                    ps_g[p:p + B, :],
                    pre[:, t, :B], pre[:, t, B:B + NB],
                    start=(t < ng), stop=(t >= KT - ng),
                    tile_position=(0, p),
                )

            # gate reduce: chained adds, each reading one psum window
            nc.vector.tensor_add(g0s[:], ps_g[gpos[0]:gpos[0] + B, :],
                                 epi[:, NOUT:NOUT + NB])
            nc.vector.tensor_add(g1s[:], ps_g[gpos[1]:gpos[1] + B, :], g0s[:])
            last = g1s
            if GATE_4WAY:
                nc.vector.tensor_add(g2s[:], ps_g[0:B, :], g1s[:])
                nc.vector.tensor_add(graw[:], ps_g[B:64, :], g2s[:])
                last = graw
            nc.scalar.activation(g[:], last[:],
                                 mybir.ActivationFunctionType.Sigmoid)
            nc.vector.max(m8[:], g[:])
            nc.vector.match_replace(rep[:], m8[:], g[:], 0.0)
            nc.vector.tensor_sub(gk[:], g[:], rep[:])

            # ---- main matmuls ----
            # two col-groups (0,0),(0,32) alternating in expected-arrival
            # order.  HSPLIT additionally halves each matmul's N to 128 and
            # emits h0-before-h1 per chunk into per-half psum tiles, so
            # half 0's epilogue and out-dma overlap half 1's last matmuls.
            order = []
            for ci in CHUNK_ORDER:
                _, lo, hi = CHUNKS[ci]
                order += list(range(lo, hi))
            assert sorted(order) == list(range(KT)), order
            chain = {0: order[0::2], 1: order[1::2]}
            half = NOUT // 2
            if HSPLIT:
                for ci in CHUNK_ORDER:
                    _, lo, hi = CHUNKS[ci]
                    for h in range(2):
                        sl = slice(h * BLK, (h + 1) * BLK)
                        for t in range(lo, hi):
                            j = t % 2
                            nc.tensor.matmul(
                                psH[h][32 * j:32 * j + B, :],
                                pre[:, t, :B], rhs[:, t, sl],
                                start=(t == chain[j][0]),
                                stop=(t == chain[j][-1]),
                                tile_position=(0, 32 * j),
                            )
            else:
                for i, t in enumerate(order):
                    j = i % 2
                    nc.tensor.matmul(
                        psA[32 * j:32 * j + B, :],
                        pre[:, t, :B], rhs[:, t, :],
                        start=(t == chain[j][0]), stop=(t == chain[j][-1]),
                        tile_position=(0, 32 * j),
                    )

            # ---- epilogue ----
            # out[:, h] = (ps0 + ps1)*gk[h] + bias via two stt passes per
            # half; each half's out-dma issues as soon as its chain is done
            # (h0 on sync, h1 on scalar so the HBM-write receipts overlap).
            epi_engs = {"v": nc.vector, "g": nc.gpsimd, "s": nc.scalar}
            for h in range(NOUT // BLK):
                sl = slice(h * BLK, (h + 1) * BLK)
                gksc = gk[:, h:h + 1]
                stt = epi_engs[EPI_ENG[h % len(EPI_ENG)]].scalar_tensor_tensor
                mul, add = mybir.AluOpType.mult, mybir.AluOpType.add
                p0 = psH[h][0:B, :] if HSPLIT else psA[0:B, sl]
                p1 = psH[h][B:64, :] if HSPLIT else psA[B:64, sl]
                stt(tmp[:, sl], p0, gksc, epi[:, sl], mul, add)
                stt(outt[:, sl], p1, gksc, tmp[:, sl], mul, add)
                if SPLIT_OUT and h == 0:
                    nc.sync.dma_start(out_d.ap()[:, 0:half], outt[:, 0:half])
            if SPLIT_OUT:
                nc.scalar.dma_start(out_d.ap()[:, half:NOUT],
                                    outt[:, half:NOUT])
            else:
                nc.sync.dma_start(out_d.ap(), outt[:])

    nc.compile()
    return nc


def _tile_major(a):
    """(D, n) -> (128, KT, n) k-tile-major contiguous."""
    n = a.shape[1]
    return np.ascontiguousarray(a.reshape(KT, 128, n).transpose(1, 0, 2))


def build_in_maps(x, gate_w, gate_b, weight, bias):
    import ml_dtypes

    x = np.asarray(x, dtype=np.float32)
    gate_w = np.asarray(gate_w, dtype=np.float32)
    gate_b = np.asarray(gate_b, dtype=np.float32)
    weight = np.asarray(weight, dtype=np.float32)
    bias = np.asarray(bias, dtype=np.float32)

    xs = (x.T / SCALE).astype(np.float16)            # (2048, 32) exact shift
    in_maps = []
    for c in range(N_CORES):
        perm = [2 * c, 2 * c + 1] + [k for k in range(NB)
                                     if k not in (2 * c, 2 * c + 1)]
        gws = (gate_w[:, perm] * SCALE).astype(np.float16)   # (2048, 16)
        pre16 = _tile_major(np.concatenate([xs, gws], axis=1).astype(np.float16))
        w_shard = weight[c * NOUT:(c + 1) * NOUT, :]          # (256, 2048)
        rhs8 = _tile_major(
            (np.ascontiguousarray(w_shard.T) * SCALE).astype(ml_dtypes.float8_e3m4))
        if SCHEME == "e":
            epi = np.empty((64, BLK + NB), dtype=np.float32)
            epi[0:B, 0:BLK] = bias[c * NOUT:c * NOUT + BLK]
            epi[B:64, 0:BLK] = bias[c * NOUT + BLK:c * NOUT + 2 * BLK]
            epi[:, BLK:] = gate_b[perm]
        else:
            epi = np.concatenate([
                np.broadcast_to(bias[c * NOUT:(c + 1) * NOUT], (B, NOUT)),
                np.broadcast_to(gate_b[perm], (B, NB)),
            ], axis=1).astype(np.float32)
        in_maps.append({
            "pre16": np.ascontiguousarray(pre16),
            "rhs8": np.ascontiguousarray(rhs8),
            "epi": np.ascontiguousarray(epi),
        })
    return in_maps


def _ensure_ntff_hook():
    """If a caller sets BASS_TRACE, run_bass_kernel_spmd imports
    antenv.axon_hooks, which is missing in this image; provide a working
    ctypes-backed stub so tracing degrades gracefully instead of raising."""
    try:
        from antenv.axon_hooks import get_axon_ntff_profile_hook  # noqa: F401
        return
    except ImportError:
        pass
    import contextlib
    import ctypes
    import types

    try:
        lib = ctypes.CDLL("/opt/axon/libaxon_pjrt.so")
        assert hasattr(lib, "axon_start_nrt_profile")
        lib.axon_start_nrt_profile.argtypes = [
            ctypes.POINTER(ctypes.c_int64), ctypes.c_size_t]
        lib.axon_start_nrt_profile.restype = ctypes.c_int64
        lib.axon_stop_nrt_profile.argtypes = [ctypes.c_char_p]
        lib.axon_stop_nrt_profile.restype = ctypes.c_int64

        @contextlib.contextmanager
        def _hook(output_dir, device_ids):
            import jax
            jax.devices()
            if device_ids:
                ids = (ctypes.c_int64 * len(device_ids))(*device_ids)
                rc = lib.axon_start_nrt_profile(ids, len(device_ids))
            else:
                rc = lib.axon_start_nrt_profile(None, 0)
            if rc != 0:
                raise RuntimeError(f"axon_start_nrt_profile rc={rc}")
            try:
                yield
            finally:
                lib.axon_stop_nrt_profile(str(output_dir).encode())

        hook = _hook
    except Exception:
        hook = None

    mod = types.ModuleType("antenv.axon_hooks")
    mod.get_axon_ntff_profile_hook = lambda: hook
    mod.set_axon_ntff_profile_hook = lambda h: None
    sys.modules["antenv.axon_hooks"] = mod


def kernel(x, gate_w, gate_b, weight, bias):
    _ensure_ntff_hook()
    from concourse.bass_utils import run_bass_kernel_spmd

    key = (tuple(map(tuple, CHUNKS)), tuple(CHUNK_ORDER),
           GATE_4WAY, PRE_SPLIT, SPLIT_OUT, PRE_ENG, WARM, HSPLIT,
           tuple(PRE_CHUNKS or []), tuple(GATE_POS), tuple(WARM_POS),
           WARM_N, EPI_ENG, EPI_LAST, SCHEME)
    if key not in _compiled:
        _compiled[key] = _build_e(key) if SCHEME == "e" else _build(key)
    nc = _compiled[key]

    in_maps = build_in_maps(x, gate_w, gate_b, weight, bias)
    res = run_bass_kernel_spmd(nc, in_maps, list(range(N_CORES)))
    out = np.concatenate([res.results[c]["out"] for c in range(N_CORES)], axis=1)
    return out.astype(np.float32)



# revision 20
# speedup vs baseline: 1.2175x; 1.2175x over previous
"""Trainium2 Bass kernel for nn_GatedBlock (moe_routing).

Math: out[b, i] = g[b, i // 128] * (x @ W.T)[b, i] + bias[i]
with g = sigmoid(x @ gate_w + gate_b), bottom-8 of 16 gates zeroed per row.

Design (vs the 24.9us f32r baseline; ~18.0-18.5us measured):
  * dtypes: one fp16 stationary x.T/64 serves both the gate matmuls (vs
    fp16 gate_w*64) and the main matmuls (vs e3m4 W*64).  The /64,*64
    scales are exact exponent shifts folded on the host.  Gate-linear
    precision (11-bit mantissa operands) matches the f32r baseline - the
    top-8 selection margin (3.4e-4) survives; main-path rel err ~1.1e-2
    vs the 2e-2 gate.  743KB/core of HBM traffic instead of 2.5MB.
  * DMA: three transfers on the sync HWDGE queue (pre16, rhs[0:8],
    rhs[8:16]) with >=1.5KB per-partition descriptor runs (the effective
    rate ceiling ~200GB/s saturates there); epi on scalar (its hoisted
    ACT table loads delay nothing late-needed); out halves split
    sync/scalar so the HBM-write receipts overlap.  No gpsimd/SWDGE: its
    descriptor-ring traffic contends with SDMA engines 7/15 and delays
    every queue's completion semaphore.
  * PE: fp16 x fp8e3m4 mixed matmuls (1 cycle/col, cheap weight loads),
    2-way column-tiled via tile_position (M=32 occupies a quarter of the
    128-wide array).  HSPLIT: each matmul covers one 128-col output half
    (N=128, 4 chains into per-half psum tiles in separate banks), emitted
    h0-before-h1 per chunk, so half 0's epilogue and out-dma overlap
    half 1's final matmuls.  Gate matmuls sit at col positions (0,64),
    (0,96), never sharing array columns with the main groups.
  * epilogue: out[:, h] = (ps0 + ps1)*gk[h] + bias as two chained
    scalar_tensor_tensor passes per half (walrus allows one PSUM input
    per DVE op; 32-partition ops may read any 32-aligned window); each
    half's out-DMA issues as soon as its chain finishes.
  * psum tiles padded to full 2KB banks so concurrent PE-writes and
    DVE-reads never share a bank.
"""

import sys

for _p in ("/opt/trn_rl_repo", "/root/.axon_site/_ro/trn_rl_repo"):
    if _p not in sys.path:
        sys.path.append(_p)

import os as _os

import numpy as np

B = 32          # batch
D = 2048        # model dim
NB = 16         # gate blocks
BLK = D // NB   # 128 output rows per gate block
N_CORES = 8
NOUT = D // N_CORES       # 256 output cols per core
KT = D // 128             # 16 k-tiles
SCALE = 64.0              # x/SCALE fp16, W*SCALE e3m4, gate_w*SCALE fp16

# rhs chunking: "s" chunks issue on sync after pre16, "c" chunks on scalar
# after epi.  Format: engine:lo:hi per chunk.  No gpsimd/SWDGE: its
# descriptor-ring traffic contends with SDMA engines 7/15 and makes every
# queue's completion semaphore late.
CHUNKS = [c.split(":") for c in
          _os.environ.get("GATED2_CHUNKS", "s:0:8,s:8:16").split(",")]
CHUNKS = [(e, int(lo), int(hi)) for e, lo, hi in CHUNKS]
# matmul emission order: chunks sorted by this key (expected arrival order)
_order_env = _os.environ.get("GATED2_ORDER", "")
CHUNK_ORDER = ([int(v) for v in _order_env.split(",")] if _order_env
               else list(range(len(CHUNKS))))
GATE_4WAY = _os.environ.get("GATED2_GATE4", "0") == "1"
PRE_SPLIT = int(_os.environ.get("GATED2_PRE_SPLIT", "1"))
SPLIT_OUT = _os.environ.get("GATED2_SPLIT_OUT", "1") == "1"
PRE_ENG = _os.environ.get("GATED2_PRE_ENG", "s")  # s=sync, c=scalar
# pre16 k-tile ranges per ring ("s:0:8,c:8:16" puts tiles 0-7 on sync and
# 8-15 on scalar).  Empty = legacy PRE_ENG/PRE_SPLIT behavior.
_pre_env = _os.environ.get("GATED2_PRE_CHUNKS", "")
PRE_CHUNKS = ([(e, int(lo), int(hi)) for e, lo, hi in
               (c.split(":") for c in _pre_env.split(","))] if _pre_env
              else None)
# gate matmul col positions (comma list).  q64/q96 keeps gates off the
# main groups q0/q32; "0,32" frees q64/q96 for long-running warm dummies.
GATE_POS = [int(v) for v in
            _os.environ.get("GATED2_GATE_POS", "64,96").split(",")]
# warm dummy col positions (cycled).  Dummies at q64/q96 never block the
# main chains at q0/q32, so the warm stream can safely outlast the DMA.
WARM_POS = [int(v) for v in
            _os.environ.get("GATED2_WARM_POS", "0").split(",")]
# where the epi (bias+gate_b) load sits in the scalar ring order
EPI_LAST = _os.environ.get("GATED2_EPI_LAST", "0") == "1"
# scheme e: gate chains duplicated on partition groups 0-31/32-63 (full
# k-sum each), mains one chain per half into a single [64,128] psum tile,
# epilogue = 2 tiny gate-col selects + ONE 64-partition DVE stt, out-dma
# halves from partition ranges 0-31 (sync) / 32-63 (scalar).
SCHEME = _os.environ.get("GATED2_SCHEME", "old")
# hoist the input dma_starts from the kernel body into the framework
# preamble (right after each engine's register setup), so the HWDGE rings
# start pulling input while the other engines still run their ~7us entry
# sequence.  Inputs are in HBM before any NEFF instruction executes, and
# the completion sems were zeroed by the previous execution's teardown.
HOIST = int(_os.environ.get("GATED2_HOIST", "0"))  # 0=off, 1=at preamble_end,
# 2=before the engine's TPBBaseLd (executes right after the entry barrier),
# 3=at the engine's stream head
# HAM warmup: dummy matmuls on a memset tile keep the PE busy from the end
# of the framework prologue until the first real data lands, so the PE
# clock-gate opens (1.2 -> 2.4 GHz) before the real matmuls run.
WARM = int(_os.environ.get("GATED2_WARM", "0"))
# free-dim of each warmup matmul (cold cost ~ N/1.2 ns each)
WARM_N = int(_os.environ.get("GATED2_WARM_N", "512"))
# epilogue engine per output half: v=vector(DVE), g=gpsimd(Pool).  "vg"
# runs half 0 on DVE and half 1 on Pool concurrently.
EPI_ENG = _os.environ.get("GATED2_EPI_ENG", "vv")
# >0: hoist the warm dummies into the PE preamble (same position codes as
# HOIST).  0 = leave them at the head of the PE body stream.
WARM_PRE = int(_os.environ.get("GATED2_WARM_PRE", "0"))
# half-split main matmuls: N=128 per matmul, 4 chains (2 col-groups x 2
# output halves) into per-half psum tiles (separate banks), emitted
# h0-before-h1 per chunk so half 0's epilogue + out-dma overlap half 1's
# final matmuls.
HSPLIT = _os.environ.get("GATED2_HSPLIT", "1") == "1"

_compiled = {}


def _build_e(key):
    """Scheme e: single [64,128] main psum (one chain per half at q0/q32),
    gate chains duplicated on both partition groups (full k-sum each), one
    64-partition epilogue stt, out halves DMA'd from partition ranges."""
    import concourse.bacc as bacc
    import concourse.tile as tile
    import concourse.mybir as mybir

    f32 = mybir.dt.float32
    f16 = mybir.dt.float16
    f8 = mybir.dt.float8e3

    nc = bacc.Bacc("TRN2", target_bir_lowering=False, debug=False,
                   num_devices=N_CORES)

    pre_d = nc.dram_tensor("pre16", [128, KT, B + NB], f16, kind="ExternalInput")
    rhs_d = nc.dram_tensor("rhs8", [128, KT, NOUT], f8, kind="ExternalInput")
    epi_d = nc.dram_tensor("epi", [64, BLK + NB], f32, kind="ExternalInput")
    out_d = nc.dram_tensor("out", [B, NOUT], f32, kind="ExternalOutput")

    with tile.TileContext(nc) as tc:
        with (
            tc.tile_pool(name="sb", bufs=1) as sb,
            tc.tile_pool(name="ps", bufs=1, space="PSUM") as psp,
        ):
            pre = sb.tile([128, KT, B + NB], f16, name="pre_sb", tag="pre_sb")
            rhs = sb.tile([128, KT, NOUT], f8, name="rhs_sb", tag="rhs_sb")
            epi = sb.tile([64, BLK + NB], f32, name="epi_sb", tag="epi_sb")
            g = sb.tile([64, NB], f32, name="g", tag="g")
            gs = sb.tile([64, NB], f32, name="gs", tag="gs")
            m8 = sb.tile([64, 8], f32, name="m8", tag="m8")
            rep = sb.tile([64, NB], f32, name="rep", tag="rep")
            gk = sb.tile([64, NB], f32, name="gk", tag="gk")
            gsel = sb.tile([64, 1], f32, name="gsel", tag="gsel")
            outt = sb.tile([64, BLK], f32, name="outt", tag="outt")

            ps_g = psp.tile([64, NB], f32, name="ps_g", tag="ps_g",
                            padded_shape=[64, 512])
            ps_m = psp.tile([64, BLK], f32, name="ps_m", tag="ps_m",
                            padded_shape=[64, 512])

            # ---- DMA issues ----
            in_dmas = []
            if PRE_CHUNKS is not None:
                for eng, lo, hi in PRE_CHUNKS:
                    e = nc.sync if eng == "s" else nc.scalar
                    in_dmas.append(
                        e.dma_start(pre[:, lo:hi, :], pre_d.ap()[:, lo:hi, :]))
            else:
                pre_e = nc.sync if PRE_ENG == "s" else nc.scalar
                in_dmas.append(pre_e.dma_start(pre[:], pre_d.ap()))
            if not EPI_LAST:
                in_dmas.append(nc.scalar.dma_start(epi[:], epi_d.ap()))
            for eng, lo, hi in CHUNKS:
                e = nc.sync if eng == "s" else nc.scalar
                in_dmas.append(
                    e.dma_start(rhs[:, lo:hi, :], rhs_d.ap()[:, lo:hi, :]))
            if EPI_LAST:
                in_dmas.append(nc.scalar.dma_start(epi[:], epi_d.ap()))

            # ---- HAM warmup: dummies at q64/q96 never touch q0/q32 ----
            warm_insts = []
            if WARM > 0:
                dum = sb.tile([128, WARM_N], f16, name="dum", tag="dum")
                ps_d = psp.tile([128, WARM_N], f32, name="ps_d", tag="ps_d",
                                padded_shape=[128, 512])
                if WARM_PRE == 0:
                    # body mode: zero the operand tile first
                    nc.gpsimd.memset(dum[:], 0.0)
                for i in range(WARM):
                    p = 64 + 32 * (i % 2)
                    warm_insts.append(
                        nc.tensor.matmul(ps_d[p:p + B, :], dum[:, :B], dum[:],
                                         start=True, stop=True,
                                         tile_position=(0, p)))

            # ---- gate matmuls: two full-k chains at q0 and q32 ----
            for t in range(KT):
                for p in (0, 32):
                    nc.tensor.matmul(
                        ps_g[p:p + B, :],
                        pre[:, t, :B], pre[:, t, B:B + NB],
                        start=(t == 0), stop=(t == KT - 1),
                        tile_position=(0, p),
                    )

            # gate pipeline on 64 partitions (same op count as 32)
            nc.vector.tensor_add(gs[:], ps_g[0:64, :], epi[:, BLK:BLK + NB])
            nc.scalar.activation(g[:], gs[:],
                                 mybir.ActivationFunctionType.Sigmoid)
            nc.vector.max(m8[:], g[:])
            nc.vector.match_replace(rep[:], m8[:], g[:], 0.0)
            nc.vector.tensor_sub(gk[:], g[:], rep[:])
            # per-half gate column select (ACT engine, off the DVE path)
            nc.scalar.copy(gsel[0:B, :], gk[0:B, 0:1])
            nc.scalar.copy(gsel[B:64, :], gk[B:64, 1:2])

            # ---- main matmuls: one chain per half, h0@q0 / h1@q32 ----
            order = []
            for ci in CHUNK_ORDER:
                _, lo, hi = CHUNKS[ci]
                order += list(range(lo, hi))
            assert sorted(order) == list(range(KT)), order
            for ci in CHUNK_ORDER:
                _, lo, hi = CHUNKS[ci]
                for t in range(lo, hi):
                    for h in range(2):
                        nc.tensor.matmul(
                            ps_m[32 * h:32 * h + B, :],
                            pre[:, t, :B],
                            rhs[:, t, h * BLK:(h + 1) * BLK],
                            start=(t == order[0]), stop=(t == order[-1]),
                            tile_position=(0, 32 * h),
                        )

            # ---- epilogue: ONE 64-partition stt, then split out-dma ----
            nc.vector.scalar_tensor_tensor(
                outt[:], ps_m[0:64, :], gsel[:, 0:1], epi[:, 0:BLK],
                mybir.AluOpType.mult, mybir.AluOpType.add)
            nc.sync.dma_start(out_d.ap()[:, 0:BLK], outt[0:B, :])
            nc.scalar.dma_start(out_d.ap()[:, BLK:NOUT], outt[B:64, :])

    if HOIST > 0:
        _hoist_input_dmas(nc, in_dmas)

    nc.compile()
    return nc


def _hoist_input_dmas(nc, in_dmas):
    """Move the input dma_start instructions from the body block into the
    preamble block, right after each issuing engine's register setup, so
    the loads issue at ~t=0.1-1.4us instead of ~7us."""
    entry = nc.main_func.blocks[0]
    per_eng = {}
    for h in in_dmas:
        inst = h.ins
        blk = None
        for b in nc.main_func.blocks:
            if any(x is inst for x in b.instructions):
                blk = b
                break
        assert blk is not None and blk is not entry, "input dma not in body"
        blk.instructions[:] = [x for x in blk.instructions if x is not inst]
        per_eng.setdefault(inst.engine, []).append(inst)
    for engkey, insts in per_eng.items():
        stream = nc.engines[engkey]
        assert stream.preamble_end is not None
        if HOIST == 1:
            idx = next(i for i, x in enumerate(entry.instructions)
                       if x is stream.preamble_end) + 1
        elif HOIST == 2:
            idx = next(i for i, x in enumerate(entry.instructions)
                       if type(x).__name__ == "InstTPBBaseLd"
                       and x.engine == engkey)
        else:
            idx = next(i for i, x in enumerate(entry.instructions)
                       if getattr(x, "engine", None) == engkey)
        entry.instructions[idx:idx] = insts


def _build(key):
    import concourse.bacc as bacc
    import concourse.tile as tile
    import concourse.mybir as mybir

    f32 = mybir.dt.float32
    f16 = mybir.dt.float16
    f8 = mybir.dt.float8e3

    nc = bacc.Bacc("TRN2", target_bir_lowering=False, debug=False,
                   num_devices=N_CORES)

    pre_d = nc.dram_tensor("pre16", [128, KT, B + NB], f16, kind="ExternalInput")
    rhs_d = nc.dram_tensor("rhs8", [128, KT, NOUT], f8, kind="ExternalInput")
    epi_d = nc.dram_tensor("epi", [B, NOUT + NB], f32, kind="ExternalInput")
    out_d = nc.dram_tensor("out", [B, NOUT], f32, kind="ExternalOutput")

    with tile.TileContext(nc) as tc:
        with (
            tc.tile_pool(name="sb", bufs=1) as sb,
            tc.tile_pool(name="ps", bufs=1, space="PSUM") as psp,
        ):
            pre = sb.tile([128, KT, B + NB], f16, name="pre_sb", tag="pre_sb")
            rhs = sb.tile([128, KT, NOUT], f8, name="rhs_sb", tag="rhs_sb")
            epi = sb.tile([B, NOUT + NB], f32, name="epi_sb", tag="epi_sb")
            g0s = sb.tile([B, NB], f32, name="g0s", tag="g0s")
            g1s = sb.tile([B, NB], f32, name="g1s", tag="g1s")
            g2s = sb.tile([B, NB], f32, name="g2s", tag="g2s")
            graw = sb.tile([B, NB], f32, name="graw", tag="graw")
            g = sb.tile([B, NB], f32, name="g", tag="g")
            m8 = sb.tile([B, 8], f32, name="m8", tag="m8")
            rep = sb.tile([B, NB], f32, name="rep", tag="rep")
            gk = sb.tile([B, NB], f32, name="gk", tag="gk")
            tmp = sb.tile([B, NOUT], f32, name="tmp", tag="tmp")
            tmp2 = sb.tile([B, NOUT], f32, name="tmp2", tag="tmp2")
            outt = sb.tile([B, NOUT], f32, name="outt", tag="outt")

            # padded to full 2KB banks: PE-writes and DVE-reads of different
            # tiles never share a bank
            ps_g = psp.tile([128, NB], f32, name="ps_g", tag="ps_g",
                            padded_shape=[128, 512])
            if HSPLIT:
                psH = [psp.tile([64, BLK], f32, name=f"psH{h}", tag=f"psH{h}",
                                padded_shape=[64, 512]) for h in range(2)]
            else:
                psA = psp.tile([64, NOUT], f32, name="psA", tag="psA",
                               padded_shape=[64, 512])

            # ---- DMA issues ----
            # sync: pre16 (first - it gates all PE work; split so the gate
            # matmuls start half a transfer earlier), then its rhs chunks.
            # scalar: epi + its rhs chunk; the hoisted ACT table load delays
            # scalar's first issue by ~1.3us, so scalar carries only
            # late-needed data.
            if PRE_CHUNKS is not None:
                for eng, lo, hi in PRE_CHUNKS:
                    e = nc.sync if eng == "s" else nc.scalar
                    e.dma_start(pre[:, lo:hi, :], pre_d.ap()[:, lo:hi, :])
            else:
                pre_e = nc.sync if PRE_ENG == "s" else nc.scalar
                if PRE_SPLIT > 1:
                    step = (KT + PRE_SPLIT - 1) // PRE_SPLIT
                    for lo in range(0, KT, step):
                        hi = min(lo + step, KT)
                        pre_e.dma_start(pre[:, lo:hi, :],
                                        pre_d.ap()[:, lo:hi, :])
                else:
                    pre_e.dma_start(pre[:], pre_d.ap())
            if not EPI_LAST:
                nc.scalar.dma_start(epi[:], epi_d.ap())
            for eng, lo, hi in CHUNKS:
                e = nc.sync if eng == "s" else nc.scalar
                e.dma_start(rhs[:, lo:hi, :], rhs_d.ap()[:, lo:hi, :])
            if EPI_LAST:
                nc.scalar.dma_start(epi[:], epi_d.ap())

            # ---- HAM warmup ----
            if WARM > 0:
                dum = sb.tile([128, WARM_N], f16, name="dum", tag="dum")
                ps_d = psp.tile([128, WARM_N], f32, name="ps_d", tag="ps_d",
                                padded_shape=[128, 512])
                nc.gpsimd.memset(dum[:], 0.0)
                for i in range(WARM):
                    p = WARM_POS[i % len(WARM_POS)]
                    nc.tensor.matmul(ps_d[p:p + B, :], dum[:, :B], dum[:],
                                     start=True, stop=True,
                                     tile_position=(0, p))

            # ---- gate matmuls ----
            # col positions (0,64),(0,96) [+(0,0),(0,32) in 4-way mode]:
            # gate output partitions sit at 64..127 so the reduce reads
            # 32-aligned windows there and main groups 0/1 are untouched.
            gpos = ([64, 96, 0, 32] if GATE_4WAY else list(GATE_POS))
            ng = len(gpos)
            for t in range(KT):
                j = t % ng
                p = gpos[j]
                nc.tensor.matmul(
                    ps_g[p:p + B, :],
                    pre[:, t, :B], pre[:, t, B:B + NB],
                    start=(t < ng), stop=(t >= KT - ng),
                    tile_position=(0, p),
                )

            # gate reduce: chained adds, each reading one psum window
            nc.vector.tensor_add(g0s[:], ps_g[gpos[0]:gpos[0] + B, :],
                                 epi[:, NOUT:NOUT + NB])
            nc.vector.tensor_add(g1s[:], ps_g[gpos[1]:gpos[1] + B, :], g0s[:])
            last = g1s
            if GATE_4WAY:
                nc.vector.tensor_add(g2s[:], ps_g[0:B, :], g1s[:])
                nc.vector.tensor_add(graw[:], ps_g[B:64, :], g2s[:])
                last = graw
            nc.scalar.activation(g[:], last[:],
                                 mybir.ActivationFunctionType.Sigmoid)
            nc.vector.max(m8[:], g[:])
            nc.vector.match_replace(rep[:], m8[:], g[:], 0.0)
            nc.vector.tensor_sub(gk[:], g[:], rep[:])

            # ---- main matmuls ----
            # two col-groups (0,0),(0,32) alternating in expected-arrival
            # order.  HSPLIT additionally halves each matmul's N to 128 and
            # emits h0-before-h1 per chunk into per-half psum tiles, so
            # half 0's epilogue and out-dma overlap half 1's last matmuls.
            order = []
            for ci in CHUNK_ORDER:
                _, lo, hi = CHUNKS[ci]
                order += list(range(lo, hi))
            assert sorted(order) == list(range(KT)), order
            chain = {0: order[0::2], 1: order[1::2]}
            half = NOUT // 2
            if HSPLIT:
                for ci in CHUNK_ORDER:
                    _, lo, hi = CHUNKS[ci]
                    for h in range(2):
                        sl = slice(h * BLK, (h + 1) * BLK)
                        for t in range(lo, hi):
                            j = t % 2
                            nc.tensor.matmul(
                                psH[h][32 * j:32 * j + B, :],
                                pre[:, t, :B], rhs[:, t, sl],
                                start=(t == chain[j][0]),
                                stop=(t == chain[j][-1]),
                                tile_position=(0, 32 * j),
                            )
            else:
                for i, t in enumerate(order):
                    j = i % 2
                    nc.tensor.matmul(
                        psA[32 * j:32 * j + B, :],
                        pre[:, t, :B], rhs[:, t, :],
                        start=(t == chain[j][0]), stop=(t == chain[j][-1]),
                        tile_position=(0, 32 * j),
                    )

            # ---- epilogue ----
            # out[:, h] = (ps0 + ps1)*gk[h] + bias via two stt passes per
            # half; each half's out-dma issues as soon as its chain is done
            # (h0 on sync, h1 on scalar so the HBM-write receipts overlap).
            epi_engs = {"v": nc.vector, "g": nc.gpsimd, "s": nc.scalar}
            for h in range(NOUT // BLK):
                sl = slice(h * BLK, (h + 1) * BLK)
                gksc = gk[:, h:h + 1]
                stt = epi_engs[EPI_ENG[h % len(EPI_ENG)]].scalar_tensor_tensor
                mul, add = mybir.AluOpType.mult, mybir.AluOpType.add
                p0 = psH[h][0:B, :] if HSPLIT else psA[0:B, sl]
                p1 = psH[h][B:64, :] if HSPLIT else psA[B:64, sl]
                stt(tmp[:, sl], p0, gksc, epi[:, sl], mul, add)
                stt(outt[:, sl], p1, gksc, tmp[:, sl], mul, add)
                if SPLIT_OUT and h == 0:
                    nc.sync.dma_start(out_d.ap()[:, 0:half], outt[:, 0:half])
            if SPLIT_OUT:
                nc.scalar.dma_start(out_d.ap()[:, half:NOUT],
                                    outt[:, half:NOUT])
            else:
                nc.sync.dma_start(out_d.ap(), outt[:])

    nc.compile()
    return nc


def _tile_major(a):
    """(D, n) -> (128, KT, n) k-tile-major contiguous."""
    n = a.shape[1]
    return np.ascontiguousarray(a.reshape(KT, 128, n).transpose(1, 0, 2))


def build_in_maps(x, gate_w, gate_b, weight, bias):
    import ml_dtypes

    x = np.asarray(x, dtype=np.float32)
    gate_w = np.asarray(gate_w, dtype=np.float32)
    gate_b = np.asarray(gate_b, dtype=np.float32)
    weight = np.asarray(weight, dtype=np.float32)
    bias = np.asarray(bias, dtype=np.float32)

    xs = (x.T / SCALE).astype(np.float16)            # (2048, 32) exact shift
    in_maps = []
    for c in range(N_CORES):
        perm = [2 * c, 2 * c + 1] + [k for k in range(NB)
                                     if k not in (2 * c, 2 * c + 1)]
        gws = (gate_w[:, perm] * SCALE).astype(np.float16)   # (2048, 16)
        pre16 = _tile_major(np.concatenate([xs, gws], axis=1).astype(np.float16))
        w_shard = weight[c * NOUT:(c + 1) * NOUT, :]          # (256, 2048)
        rhs8 = _tile_major(
            (np.ascontiguousarray(w_shard.T) * SCALE).astype(ml_dtypes.float8_e3m4))
        if SCHEME == "e":
            epi = np.empty((64, BLK + NB), dtype=np.float32)
            epi[0:B, 0:BLK] = bias[c * NOUT:c * NOUT + BLK]
            epi[B:64, 0:BLK] = bias[c * NOUT + BLK:c * NOUT + 2 * BLK]
            epi[:, BLK:] = gate_b[perm]
        else:
            epi = np.concatenate([
                np.broadcast_to(bias[c * NOUT:(c + 1) * NOUT], (B, NOUT)),
                np.broadcast_to(gate_b[perm], (B, NB)),
            ], axis=1).astype(np.float32)
        in_maps.append({
            "pre16": np.ascontiguousarray(pre16),
            "rhs8": np.ascontiguousarray(rhs8),
            "epi": np.ascontiguousarray(epi),
        })
    return in_maps


def _ensure_ntff_hook():
    """If a caller sets BASS_TRACE, run_bass_kernel_spmd imports
    antenv.axon_hooks, which is missing in this image; provide a working
    ctypes-backed stub so tracing degrades gracefully instead of raising."""
    try:
        from antenv.axon_hooks import get_axon_ntff_profile_hook  # noqa: F401
        return
    except ImportError:
        pass
    import contextlib
    import ctypes
    import types

    try:
        lib = ctypes.CDLL("/opt/axon/libaxon_pjrt.so")
        assert hasattr(lib, "axon_start_nrt_profile")
        lib.axon_start_nrt_profile.argtypes = [
            ctypes.POINTER(ctypes.c_int64), ctypes.c_size_t]
        lib.axon_start_nrt_profile.restype = ctypes.c_int64
        lib.axon_stop_nrt_profile.argtypes = [ctypes.c_char_p]
        lib.axon_stop_nrt_profile.restype = ctypes.c_int64

        @contextlib.contextmanager
        def _hook(output_dir, device_ids):
            import jax
            jax.devices()
            if device_ids:
                ids = (ctypes.c_int64 * len(device_ids))(*device_ids)
                rc = lib.axon_start_nrt_profile(ids, len(device_ids))
            else:
                rc = lib.axon_start_nrt_profile(None, 0)
            if rc != 0:
                raise RuntimeError(f"axon_start_nrt_profile rc={rc}")
            try:
                yield
            finally:
                lib.axon_stop_nrt_profile(str(output_dir).encode())

        hook = _hook
    except Exception:
        hook = None

    mod = types.ModuleType("antenv.axon_hooks")
    mod.get_axon_ntff_profile_hook = lambda: hook
    mod.set_axon_ntff_profile_hook = lambda h: None
    sys.modules["antenv.axon_hooks"] = mod


def kernel(x, gate_w, gate_b, weight, bias):
    _ensure_ntff_hook()
    from concourse.bass_utils import run_bass_kernel_spmd

    key = (tuple(map(tuple, CHUNKS)), tuple(CHUNK_ORDER),
           GATE_4WAY, PRE_SPLIT, SPLIT_OUT, PRE_ENG, WARM, HSPLIT,
           tuple(PRE_CHUNKS or []), tuple(GATE_POS), tuple(WARM_POS),
           WARM_N, EPI_ENG, EPI_LAST, SCHEME)
    if key not in _compiled:
        _compiled[key] = _build_e(key) if SCHEME == "e" else _build(key)
    nc = _compiled[key]

    in_maps = build_in_maps(x, gate_w, gate_b, weight, bias)
    res = run_bass_kernel_spmd(nc, in_maps, list(range(N_CORES)))
    out = np.concatenate([res.results[c]["out"] for c in range(N_CORES)], axis=1)
    return out.astype(np.float32)

